# revision 2
# baseline (speedup 1.0000x reference)
"""Trainium2 Bass kernel for nn_Attention_44504451121208.

Dual-stream (x / x_hsi) 12-head attention block:
  qkv -> [template-template attn (shared), search-all attn per stream] -> proj.

Strategy: data-parallel over batch B=64 across 8 NeuronCores (8 batches/core,
no collectives). All matmuls in bf16 (fp32 accumulation in PSUM).

Layout notes (per core):
 - Tokens are reordered internally to [search 256 | template 128] so the
   template-key chunk is a single contraction tile and the template-query
   columns ride along the search columns in one matmul.
 - x is cast to bf16, staged to DRAM, and read back with XBAR DMA-transpose
   to give x^T [C, N] tiles (contraction over C needs C on partitions).
 - q^T,k^T computed head-major [C, N] (weights stationary); v computed
   token-major [N, C] (x^T stationary) with a ones-column appended per head
   so the attention-value matmul also emits softmax denominators (row 64).
 - Scores are computed key-major S^T = k @ q^T; exp on ScalarE (logits are
   tiny: |s|<~3, so no max subtraction, matching softmax semantics exactly
   up to fp rounding); O^T = (v_aug)^T @ exp(S^T) accumulates over key chunks.
 - Normalization: reciprocal of the denominator row, gpsimd partition
   broadcast, one fused multiply during the PSUM->SBUF evacuation.
 - proj consumes O^T head-major directly (stationary), emits token-major
   tiles, adds bias broadcast, DMAs straight to the outputs. The template
   block is computed once and written to both outputs.
"""

import sys

sys.path.insert(0, "/opt/trn_rl_repo")

import numpy as np

B, N, C = 64, 384, 768
H, DH = 12, 64
LT, LS = 128, 256  # template / search token counts
NCORES = 8
BL = B // NCORES  # batches per core
CK = C // 128  # contraction chunks
NT = N // 128  # token tiles
C3 = 3 * C

_CACHE = {}


def _build_program():
    import concourse.tile as tile
    from concourse import bacc, library_config
    import concourse.mybir as mybir

    dt = mybir.dt
    BF, F32 = dt.bfloat16, dt.float32
    Exp = mybir.ActivationFunctionType.Exp

    nc = bacc.Bacc("TRN2", target_bir_lowering=False, debug=False)

    x_in = nc.dram_tensor("x", [BL, N, C], F32, kind="ExternalInput")
    xh_in = nc.dram_tensor("x_hsi", [BL, N, C], F32, kind="ExternalInput")
    qkvw_in = nc.dram_tensor("qkv_w", [C3, C], F32, kind="ExternalInput")
    projw_in = nc.dram_tensor("proj_w", [C, C], F32, kind="ExternalInput")
    projb_in = nc.dram_tensor("proj_b", [C], F32, kind="ExternalInput")
    out0 = nc.dram_tensor("out", [BL, N, C], F32, kind="ExternalOutput")
    out1 = nc.dram_tensor("out_hsi", [BL, N, C], F32, kind="ExternalOutput")

    with tile.TileContext(nc) as tc:
        with tc.tile_critical():
            nc.gpsimd.load_library(library_config.attn)

        import contextlib

        stack = contextlib.ExitStack()
        with stack:
            const = stack.enter_context(tc.tile_pool(name="const", bufs=1))
            stage = stack.enter_context(tc.tile_pool(name="stage", bufs=6))
            dram = stack.enter_context(tc.tile_pool(name="dram", bufs=4, space="DRAM"))
            wdram = stack.enter_context(tc.tile_pool(name="wdram", bufs=1, space="DRAM"))
            xtp = stack.enter_context(tc.tile_pool(name="xtp", bufs=12))
            qkp = stack.enter_context(tc.tile_pool(name="qkp", bufs=36))
            vbp = stack.enter_context(tc.tile_pool(name="vbp", bufs=9))
            atp = stack.enter_context(tc.tile_pool(name="atp", bufs=3))
            recp = stack.enter_context(tc.tile_pool(name="recp", bufs=4))
            obp = stack.enter_context(tc.tile_pool(name="obp", bufs=12))
            osbp = stack.enter_context(tc.tile_pool(name="osbp", bufs=4))
            psmm = stack.enter_context(tc.tile_pool(name="psmm", bufs=3, space="PSUM"))
            pss = stack.enter_context(tc.tile_pool(name="pss", bufs=3, space="PSUM"))
            pso = stack.enter_context(tc.tile_pool(name="pso", bufs=2, space="PSUM"))

            # ---------------- weights ----------------
            # stage bf16 copies of qkv_w / proj_w to DRAM, then XBAR-transpose
            # them back so the contraction dim (input C) is on partitions.
            qkv_wb = wdram.tile([C3, C], BF, tag="qkv_wb")
            proj_wb = wdram.tile([C, C], BF, tag="proj_wb")
            for r in range(C3 // 128):
                ld = stage.tile([128, C], F32, tag="stageld")
                nc.sync.dma_start(ld[:], qkvw_in[r * 128 : (r + 1) * 128, :])
                cs = stage.tile([128, C], BF, tag="stagecs")
                nc.vector.tensor_copy(cs[:], ld[:])
                nc.sync.dma_start(qkv_wb[r * 128 : (r + 1) * 128, :], cs[:])
            for r in range(C // 128):
                ld = stage.tile([128, C], F32, tag="stageld")
                nc.sync.dma_start(ld[:], projw_in[r * 128 : (r + 1) * 128, :])
                cs = stage.tile([128, C], BF, tag="stagecs")
                nc.vector.tensor_copy(cs[:], ld[:])
                nc.sync.dma_start(proj_wb[r * 128 : (r + 1) * 128, :], cs[:])

            wt = []  # qkv_w^T chunks: wt[ci] = [128 (C rows ci), 2304]
            wpt = []  # proj_w^T chunks: wpt[ci] = [128, 768]
            for ci in range(CK):
                t = const.tile([128, C3], BF, tag=f"wt{ci}")
                nc.sync.dma_start_transpose(t[:], qkv_wb[:, ci * 128 : (ci + 1) * 128])
                wt.append(t)
            for ci in range(CK):
                t = const.tile([128, C], BF, tag=f"wpt{ci}")
                nc.sync.dma_start_transpose(t[:], proj_wb[:, ci * 128 : (ci + 1) * 128])
                wpt.append(t)

            bias1 = const.tile([1, C], F32, tag="bias1")
            nc.sync.dma_start(bias1[:], projb_in[:].unsqueeze(0))
            bias_bc = const.tile([128, C], F32, tag="bias_bc")
            nc.gpsimd.partition_broadcast(bias_bc[:], bias1[:])

            # ---------------- per-batch pipeline ----------------
            def stage_matrix(src, b):
                """cast one [N, C] fp32 matrix to bf16 in DRAM, tokens
                reordered to [search | template]; return x^T SBUF tiles."""
                xb = dram.tile([N, C], BF, tag="xb")
                for t in range(NT):
                    ld = stage.tile([128, C], F32, tag="stageld")
                    nc.sync.dma_start(ld[:], src[b, t * 128 : (t + 1) * 128, :])
                    cs = stage.tile([128, C], BF, tag="stagecs")
                    nc.vector.tensor_copy(cs[:], ld[:])
                    it = (t + 2) % NT
                    nc.sync.dma_start(xb[it * 128 : (it + 1) * 128, :], cs[:])
                xt = []
                for ci in range(CK):
                    t = xtp.tile([128, N], BF, tag="xt")
                    nc.sync.dma_start_transpose(t[:], xb[:, ci * 128 : (ci + 1) * 128])
                    xt.append(t)
                return xt

            def qkv_matrix(xt, hsi):
                """q^T,k^T head-major tiles + v token-major (65-strided heads
                with a trailing ones column per head)."""
                qk = []
                for m in range(12):
                    fm = 256 if (hsi and m < 6) else 384  # hsi q: search only
                    ps = psmm.tile([128, 384], F32, tag="mm")
                    for ci in range(CK):
                        nc.tensor.matmul(
                            ps[:, :fm],
                            wt[ci][:, m * 128 : (m + 1) * 128],
                            xt[ci][:, :fm],
                            start=(ci == 0),
                            stop=(ci == CK - 1),
                        )
                    t = qkp.tile([128, 384], BF, tag="qk")
                    nc.vector.tensor_copy(t[:, :fm], ps[:, :fm])
                    qk.append(t)
                vb = []
                for it in range(NT):
                    t = vbp.tile([128, 12 * 65], BF, tag="vb")
                    nc.vector.memset(t[:], 1.0)
                    psa = psmm.tile([128, 384], F32, tag="mm")
                    psb = psmm.tile([128, 384], F32, tag="mm")
                    for ci in range(CK):
                        lhs = xt[ci][:, it * 128 : (it + 1) * 128]
                        nc.tensor.matmul(
                            psa[:], lhs, wt[ci][:, 1536:1920],
                            start=(ci == 0), stop=(ci == CK - 1),
                        )
                        nc.tensor.matmul(
                            psb[:], lhs, wt[ci][:, 1920:2304],
                            start=(ci == 0), stop=(ci == CK - 1),
                        )
                    v3 = t[:].rearrange("p (h e) -> p h e", e=65)
                    nc.vector.tensor_copy(
                        v3[:, 0:6, 0:64], psa[:].rearrange("p (h e) -> p h e", e=64)
                    )
                    nc.vector.tensor_copy(
                        v3[:, 6:12, 0:64], psb[:].rearrange("p (h e) -> p h e", e=64)
                    )
                    vb.append(t)
                return qk, vb

            def attend(qk, vb, h, hsi, obuf):
                """one head, one stream: S^T -> exp -> O^T(+denom) -> normalize
                into obuf columns."""
                po = 64 * (h % 2)
                k_h = qk[6 + h // 2][po : po + 64, :]
                nq = 256 if hsi else 384
                q_h = qk[h // 2][po : po + 64, 0:nq]

                at = atp.tile([128, 3 * nq], BF, tag="ath" if hsi else "at")
                at3 = at[:].rearrange("p (c q) -> p c q", q=nq)
                for ck in (2, 0, 1):
                    fq = nq if ck == 2 else 256
                    sp = pss.tile([128, 384], F32, tag="s")
                    nc.tensor.matmul(
                        sp[:, :fq],
                        k_h[:, ck * 128 : (ck + 1) * 128],
                        q_h[:, :fq],
                        start=True,
                        stop=True,
                    )
                    nc.scalar.activation(at3[:, ck, :fq], sp[:, :fq], Exp, scale=0.125)

                op = pso.tile([65, 384], F32, tag="o")
                for i, ck in enumerate((2, 0, 1)):
                    fq = nq if ck == 2 else 256
                    nc.tensor.matmul(
                        op[:, :fq],
                        vb[ck][:, h * 65 : (h + 1) * 65],
                        at3[:, ck, :fq],
                        start=(i == 0),
                        stop=(i == 2),
                        skip_group_check=True,
                    )
                rec = recp.tile([1, 384], F32, tag="rec")
                nc.vector.reciprocal(rec[:, :nq], op[64:65, :nq])
                rbc = recp.tile([64, 384], F32, tag="rbc")
                nc.gpsimd.partition_broadcast(rbc[:, :nq], rec[:, :nq])
                co = 384 if hsi else 0
                nc.vector.tensor_mul(
                    obuf[h // 2][po : po + 64, co : co + nq],
                    op[0:64, :nq],
                    rbc[:, :nq],
                )

            # output column ranges of the 5 proj tiles (internal order):
            # 0: main search 0:128   -> out[b, 128:256]
            # 1: main search 128:256 -> out[b, 256:384]
            # 2: template (shared)   -> out[b, 0:128] and out_hsi[b, 0:128]
            # 3: hsi search 0:128    -> out_hsi[b, 128:256]
            # 4: hsi search 128:256  -> out_hsi[b, 256:384]
            def proj(obuf, b):
                targets = [
                    [(out0, 128)],
                    [(out0, 256)],
                    [(out0, 0), (out1, 0)],
                    [(out1, 128)],
                    [(out1, 256)],
                ]
                for tt in range(5):
                    psa = psmm.tile([128, 384], F32, tag="mm")
                    psb = psmm.tile([128, 384], F32, tag="mm")
                    for ci in range(CK):
                        lhs = obuf[ci][:, tt * 128 : (tt + 1) * 128]
                        nc.tensor.matmul(
                            psa[:], lhs, wpt[ci][:, 0:384],
                            start=(ci == 0), stop=(ci == CK - 1),
                        )
                        nc.tensor.matmul(
                            psb[:], lhs, wpt[ci][:, 384:768],
                            start=(ci == 0), stop=(ci == CK - 1),
                        )
                    ob = osbp.tile([128, C], F32, tag="outsb")
                    nc.vector.tensor_add(ob[:, 0:384], psa[:], bias_bc[:, 0:384])
                    nc.vector.tensor_add(ob[:, 384:768], psb[:], bias_bc[:, 384:768])
                    for dst, row in targets[tt]:
                        nc.sync.dma_start(dst[b, row : row + 128, :], ob[:])

            for b in range(BL):
                xt_m = stage_matrix(x_in, b)
                xt_h = stage_matrix(xh_in, b)
                qk_m, vb_m = qkv_matrix(xt_m, hsi=False)
                qk_h, vb_h = qkv_matrix(xt_h, hsi=True)
                obuf = [
                    obp.tile([128, 640], BF, tag="obuf", name=f"obuf_{b}_{j}")
                    for j in range(CK)
                ]
                for h in range(H):
                    attend(qk_m, vb_m, h, False, obuf)
                    attend(qk_h, vb_h, h, True, obuf)
                proj(obuf, b)

    nc.compile()
    return nc


def _get_program():
    if "nc" not in _CACHE:
        _CACHE["nc"] = _build_program()
    return _CACHE["nc"]


def kernel(x, x_hsi, qkv_w, proj_w, proj_b, t_h=8, t_w=8, s_h=16, s_w=16,
           num_heads=12, **_ignored):
    from concourse.bass_utils import run_bass_kernel_spmd

    nc = _get_program()
    x = np.asarray(x, dtype=np.float32)
    x_hsi = np.asarray(x_hsi, dtype=np.float32)
    qkv_w = np.asarray(qkv_w, dtype=np.float32)
    proj_w = np.asarray(proj_w, dtype=np.float32)
    proj_b = np.asarray(proj_b, dtype=np.float32)

    core_ids = list(range(NCORES))
    in_maps = [
        {
            "x": x[c * BL : (c + 1) * BL],
            "x_hsi": x_hsi[c * BL : (c + 1) * BL],
            "qkv_w": qkv_w,
            "proj_w": proj_w,
            "proj_b": proj_b,
        }
        for c in core_ids
    ]
    res = run_bass_kernel_spmd(nc, in_maps, core_ids)
    out = np.concatenate([res.results[c]["out"] for c in core_ids], axis=0)
    out_hsi = np.concatenate([res.results[c]["out_hsi"] for c in core_ids], axis=0)
    return out, out_hsi


# revision 5
# speedup vs baseline: 1.2344x; 1.2344x over previous
"""Trainium2 Bass kernel for nn_Attention_44504451121208.

Dual-stream (x / x_hsi) 12-head attention block:
  qkv -> [template-template attn (shared), search-all attn per stream] -> proj.

Strategy: data-parallel over batch B=64 across 8 NeuronCores (8 batches/core,
no collectives). All matmuls in bf16 (fp32 accumulation in PSUM).

Layout notes (per core):
 - Tokens are reordered internally to [search 256 | template 128] so the
   template-key chunk is a single contraction tile and the template-query
   columns ride along the search columns in one matmul.
 - x is cast to bf16, staged to DRAM, and read back with XBAR DMA-transpose
   to give x^T [C, N] tiles (contraction over C needs C on partitions).
 - q^T,k^T computed head-major [C, N] (weights stationary); v computed
   token-major [N, C] (x^T stationary) with a ones-column appended per head
   so the attention-value matmul also emits softmax denominators (row 64).
 - Scores are computed key-major S^T = k @ q^T; exp on ScalarE (logits are
   tiny: |s|<~3, so no max subtraction, matching softmax semantics exactly
   up to fp rounding); O^T = (v_aug)^T @ exp(S^T) accumulates over key chunks.
 - Normalization: reciprocal of the denominator row, gpsimd partition
   broadcast, one fused multiply during the PSUM->SBUF evacuation.
 - proj consumes O^T head-major directly (stationary), emits token-major
   tiles, adds bias broadcast, DMAs straight to the outputs. The template
   block is computed once and written to both outputs.
"""

import sys

sys.path.insert(0, "/opt/trn_rl_repo")

import numpy as np

B, N, C = 64, 384, 768
H, DH = 12, 64
LT, LS = 128, 256  # template / search token counts
NCORES = 8
BL = B // NCORES  # batches per core
CK = C // 128  # contraction chunks
NT = N // 128  # token tiles
C3 = 3 * C

_CACHE = {}


def _build_program(variant="default"):
    import concourse.tile as tile
    from concourse import bacc, library_config
    import concourse.mybir as mybir

    dt = mybir.dt
    BF, F32 = dt.bfloat16, dt.float32
    Exp = mybir.ActivationFunctionType.Exp

    nc = bacc.Bacc("TRN2", target_bir_lowering=False, debug=False)

    x_in = nc.dram_tensor("x", [BL, N, C], F32, kind="ExternalInput")
    xh_in = nc.dram_tensor("x_hsi", [BL, N, C], F32, kind="ExternalInput")
    qkvw_in = nc.dram_tensor("qkv_w", [C3, C], F32, kind="ExternalInput")
    projw_in = nc.dram_tensor("proj_w", [C, C], F32, kind="ExternalInput")
    projb_in = nc.dram_tensor("proj_b", [C], F32, kind="ExternalInput")
    out0 = nc.dram_tensor("out", [BL, N, C], F32, kind="ExternalOutput")
    out1 = nc.dram_tensor("out_hsi", [BL, N, C], F32, kind="ExternalOutput")

    with tile.TileContext(nc) as tc:
        with tc.tile_critical():
            nc.gpsimd.load_library(library_config.attn)

        import contextlib

        stack = contextlib.ExitStack()
        with stack:
            const = stack.enter_context(tc.tile_pool(name="const", bufs=1))
            stage = stack.enter_context(tc.tile_pool(name="stage", bufs=6))
            dram = stack.enter_context(tc.tile_pool(name="dram", bufs=4, space="DRAM"))
            wdram = stack.enter_context(tc.tile_pool(name="wdram", bufs=1, space="DRAM"))
            xtp = stack.enter_context(tc.tile_pool(name="xtp", bufs=12))
            qkp = stack.enter_context(tc.tile_pool(name="qkp", bufs=36))
            vbp = stack.enter_context(tc.tile_pool(name="vbp", bufs=9))
            atp = stack.enter_context(tc.tile_pool(name="atp", bufs=3))
            recp = stack.enter_context(tc.tile_pool(name="recp", bufs=4))
            obp = stack.enter_context(tc.tile_pool(name="obp", bufs=12))
            osbp = stack.enter_context(tc.tile_pool(name="osbp", bufs=4))
            psmm = stack.enter_context(tc.tile_pool(name="psmm", bufs=3, space="PSUM"))
            pss = stack.enter_context(tc.tile_pool(name="pss", bufs=3, space="PSUM"))
            pso = stack.enter_context(tc.tile_pool(name="pso", bufs=2, space="PSUM"))

            # ---------------- weights ----------------
            # stage bf16 copies of qkv_w / proj_w to DRAM, then XBAR-transpose
            # them back so the contraction dim (input C) is on partitions.
            qkv_wb = wdram.tile([C3, C], BF, tag="qkv_wb")
            proj_wb = wdram.tile([C, C], BF, tag="proj_wb")
            for r in range(C3 // 128):
                ld = stage.tile([128, C], F32, tag="stageld")
                nc.sync.dma_start(ld[:], qkvw_in[r * 128 : (r + 1) * 128, :])
                cs = stage.tile([128, C], BF, tag="stagecs")
                nc.vector.tensor_copy(cs[:], ld[:])
                nc.sync.dma_start(qkv_wb[r * 128 : (r + 1) * 128, :], cs[:])
            for r in range(C // 128):
                ld = stage.tile([128, C], F32, tag="stageld")
                nc.sync.dma_start(ld[:], projw_in[r * 128 : (r + 1) * 128, :])
                cs = stage.tile([128, C], BF, tag="stagecs")
                nc.vector.tensor_copy(cs[:], ld[:])
                nc.sync.dma_start(proj_wb[r * 128 : (r + 1) * 128, :], cs[:])

            wt = []  # qkv_w^T chunks: wt[ci] = [128 (C rows ci), 2304]
            wpt = []  # proj_w^T chunks: wpt[ci] = [128, 768]
            for ci in range(CK):
                t = const.tile([128, C3], BF, tag=f"wt{ci}")
                nc.sync.dma_start_transpose(t[:], qkv_wb[:, ci * 128 : (ci + 1) * 128])
                wt.append(t)
            for ci in range(CK):
                t = const.tile([128, C], BF, tag=f"wpt{ci}")
                nc.sync.dma_start_transpose(t[:], proj_wb[:, ci * 128 : (ci + 1) * 128])
                wpt.append(t)

            bias1 = const.tile([1, C], F32, tag="bias1")
            nc.sync.dma_start(bias1[:], projb_in[:].unsqueeze(0))
            bias_bc = const.tile([128, C], F32, tag="bias_bc")
            nc.gpsimd.partition_broadcast(bias_bc[:], bias1[:])

            # ---------------- per-batch pipeline ----------------
            def stage_matrix(src, b):
                """cast one [N, C] fp32 matrix to bf16 in DRAM, tokens
                reordered to [search | template]; return x^T SBUF tiles."""
                xb = dram.tile([N, C], BF, tag="xb")
                for t in range(NT):
                    ld = stage.tile([128, C], F32, tag="stageld")
                    nc.sync.dma_start(ld[:], src[b, t * 128 : (t + 1) * 128, :])
                    cs = stage.tile([128, C], BF, tag="stagecs")
                    nc.vector.tensor_copy(cs[:], ld[:])
                    it = (t + 2) % NT
                    nc.sync.dma_start(xb[it * 128 : (it + 1) * 128, :], cs[:])
                xt = []
                for ci in range(CK):
                    t = xtp.tile([128, N], BF, tag="xt")
                    nc.sync.dma_start_transpose(t[:], xb[:, ci * 128 : (ci + 1) * 128])
                    xt.append(t)
                return xt

            def qkv_matrix(xt, hsi):
                """q^T,k^T head-major tiles + v token-major (65-strided heads
                with a trailing ones column per head)."""
                qk = []
                for m in range(12):
                    fm = 256 if (hsi and m < 6) else 384  # hsi q: search only
                    ps = psmm.tile([128, 384], F32, tag="mm")
                    for ci in range(CK):
                        nc.tensor.matmul(
                            ps[:, :fm],
                            wt[ci][:, m * 128 : (m + 1) * 128],
                            xt[ci][:, :fm],
                            start=(ci == 0),
                            stop=(ci == CK - 1),
                        )
                    t = qkp.tile([128, 384], BF, tag="qk")
                    nc.vector.tensor_copy(t[:, :fm], ps[:, :fm])
                    qk.append(t)
                vb = []
                for it in range(NT):
                    t = vbp.tile([128, 12 * 65], BF, tag="vb")
                    nc.vector.memset(t[:], 1.0)
                    psa = psmm.tile([128, 384], F32, tag="mm")
                    psb = psmm.tile([128, 384], F32, tag="mm")
                    for ci in range(CK):
                        lhs = xt[ci][:, it * 128 : (it + 1) * 128]
                        nc.tensor.matmul(
                            psa[:], lhs, wt[ci][:, 1536:1920],
                            start=(ci == 0), stop=(ci == CK - 1),
                        )
                        nc.tensor.matmul(
                            psb[:], lhs, wt[ci][:, 1920:2304],
                            start=(ci == 0), stop=(ci == CK - 1),
                        )
                    v3 = t[:].rearrange("p (h e) -> p h e", e=65)
                    nc.vector.tensor_copy(
                        v3[:, 0:6, 0:64], psa[:].rearrange("p (h e) -> p h e", e=64)
                    )
                    nc.vector.tensor_copy(
                        v3[:, 6:12, 0:64], psb[:].rearrange("p (h e) -> p h e", e=64)
                    )
                    vb.append(t)
                return qk, vb

            def attend(qk, vb, h, hsi, obuf):
                """one head, one stream: S^T -> exp -> O^T(+denom) -> normalize
                into obuf columns."""
                po = 64 * (h % 2)
                k_h = qk[6 + h // 2][po : po + 64, :]
                nq = 256 if hsi else 384
                q_h = qk[h // 2][po : po + 64, 0:nq]

                at = atp.tile([128, 3 * nq], BF, tag="ath" if hsi else "at")
                at3 = at[:].rearrange("p (c q) -> p c q", q=nq)
                for ck in (2, 0, 1):
                    fq = nq if ck == 2 else 256
                    sp = pss.tile([128, 384], F32, tag="s")
                    nc.tensor.matmul(
                        sp[:, :fq],
                        k_h[:, ck * 128 : (ck + 1) * 128],
                        q_h[:, :fq],
                        start=True,
                        stop=True,
                    )
                    nc.scalar.activation(at3[:, ck, :fq], sp[:, :fq], Exp, scale=0.125)

                op = pso.tile([65, 384], F32, tag="o")
                for i, ck in enumerate((2, 0, 1)):
                    fq = nq if ck == 2 else 256
                    nc.tensor.matmul(
                        op[:, :fq],
                        vb[ck][:, h * 65 : (h + 1) * 65],
                        at3[:, ck, :fq],
                        start=(i == 0),
                        stop=(i == 2),
                        skip_group_check=True,
                    )
                co = 384 if hsi else 0
                if variant == "nonorm":
                    nc.vector.tensor_copy(
                        obuf[h // 2][po : po + 64, co : co + nq], op[0:64, :nq]
                    )
                else:
                    rec = recp.tile([1, 384], F32, tag="rec")
                    nc.vector.reciprocal(rec[:, :nq], op[64:65, :nq])
                    rbc = recp.tile([64, 384], F32, tag="rbc")
                    nc.gpsimd.partition_broadcast(rbc[:, :nq], rec[:, :nq])
                    nc.vector.tensor_mul(
                        obuf[h // 2][po : po + 64, co : co + nq],
                        op[0:64, :nq],
                        rbc[:, :nq],
                    )

            # output column ranges of the 5 proj tiles (internal order):
            # 0: main search 0:128   -> out[b, 128:256]
            # 1: main search 128:256 -> out[b, 256:384]
            # 2: template (shared)   -> out[b, 0:128] and out_hsi[b, 0:128]
            # 3: hsi search 0:128    -> out_hsi[b, 128:256]
            # 4: hsi search 128:256  -> out_hsi[b, 256:384]
            def proj(obuf, b):
                targets = [
                    [(out0, 128)],
                    [(out0, 256)],
                    [(out0, 0), (out1, 0)],
                    [(out1, 128)],
                    [(out1, 256)],
                ]
                for tt in range(5):
                    psa = psmm.tile([128, 384], F32, tag="mm")
                    psb = psmm.tile([128, 384], F32, tag="mm")
                    for ci in range(CK):
                        lhs = obuf[ci][:, tt * 128 : (tt + 1) * 128]
                        nc.tensor.matmul(
                            psa[:], lhs, wpt[ci][:, 0:384],
                            start=(ci == 0), stop=(ci == CK - 1),
                        )
                        nc.tensor.matmul(
                            psb[:], lhs, wpt[ci][:, 384:768],
                            start=(ci == 0), stop=(ci == CK - 1),
                        )
                    ob = osbp.tile([128, C], F32, tag="outsb")
                    nc.vector.tensor_add(ob[:, 0:384], psa[:], bias_bc[:, 0:384])
                    nc.vector.tensor_add(ob[:, 384:768], psb[:], bias_bc[:, 384:768])
                    for dst, row in targets[tt]:
                        nc.sync.dma_start(dst[b, row : row + 128, :], ob[:])

            for b in range(BL):
                xt_m = stage_matrix(x_in, b)
                xt_h = stage_matrix(xh_in, b)
                qk_m, vb_m = qkv_matrix(xt_m, hsi=False)
                qk_h, vb_h = qkv_matrix(xt_h, hsi=True)
                obuf = [
                    obp.tile([128, 640], BF, tag="obuf", name=f"obuf_{b}_{j}")
                    for j in range(CK)
                ]
                for h in range(H):
                    attend(qk_m, vb_m, h, False, obuf)
                    attend(qk_h, vb_h, h, True, obuf)
                proj(obuf, b)

    nc.compile()
    return nc


def _get_program(variant="default"):
    if variant not in _CACHE:
        _CACHE[variant] = _build_program(variant)
    return _CACHE[variant]


def kernel(x, x_hsi, qkv_w, proj_w, proj_b, t_h=8, t_w=8, s_h=16, s_w=16,
           num_heads=12, **_ignored):
    from concourse.bass_utils import run_bass_kernel_spmd

    nc = _get_program()
    x = np.asarray(x, dtype=np.float32)
    x_hsi = np.asarray(x_hsi, dtype=np.float32)
    qkv_w = np.asarray(qkv_w, dtype=np.float32)
    proj_w = np.asarray(proj_w, dtype=np.float32)
    proj_b = np.asarray(proj_b, dtype=np.float32)

    core_ids = list(range(NCORES))
    in_maps = [
        {
            "x": x[c * BL : (c + 1) * BL],
            "x_hsi": x_hsi[c * BL : (c + 1) * BL],
            "qkv_w": qkv_w,
            "proj_w": proj_w,
            "proj_b": proj_b,
        }
        for c in core_ids
    ]
    res = run_bass_kernel_spmd(nc, in_maps, core_ids)
    out = np.concatenate([res.results[c]["out"] for c in core_ids], axis=0)
    out_hsi = np.concatenate([res.results[c]["out_hsi"] for c in core_ids], axis=0)
    return out, out_hsi
